# revision 5
# baseline (speedup 1.0000x reference)
"""GQA attention (S=2048, D=2048, 32 q-heads / 8 kv-heads, rope, causal) on 8
Trainium2 NeuronCores, tensor-parallel over heads (1 kv head + 4 q heads per
core), AllToAll re-shard before the output projection, row-sharded output.

Self-contained: takes full inputs, shards on host, runs one SPMD Bass/Tile
kernel via run_bass_kernel_spmd, reassembles the full output.

Layout notes (everything on-chip lives in the transposed/"T" domain):
 - xT (D,S) host-transposed so the contraction dim D is the SBUF partition dim.
 - q/k weights are column-permuted per head (evens then odds) so rope becomes
   ops on contiguous 32-row blocks; scores are permutation-invariant.
 - scoresT[s,q] = kT.T @ qT per 128-row s-block; softmax denominators come for
   free from a ones-column appended to V (row 64 of the PV psum).
 - softmax skips the max-subtraction: scores*0.125 ~ N(0,1), exp is safe in f32.
 - causal masking: s-blocks strictly above the diagonal are skipped, the
   diagonal 128x128 sub-block gets mask[:128,:128].T added pre-exp (all
   diagonal blocks of a causal mask are identical), below-diagonal sub-block
   columns inside partial tiles are zero-filled in probs.
"""
import os
import sys
from contextlib import ExitStack

import numpy as np

try:
    import concourse.bass as bass  # noqa: F401
except ImportError:  # platform tree not on sys.path in a fresh dir
    sys.path.insert(0, "/opt/trn_rl_repo")
    import concourse.bass as bass  # noqa: F401

import concourse.mybir as mybir
from concourse import bacc, bass_utils, tile
from concourse.masks import make_identity

F32 = mybir.dt.float32
F32R = mybir.dt.float32r
AF = mybir.ActivationFunctionType

S = 2048          # sequence length
D = 2048          # model dim
HD = 64           # head dim
N_CORES = 8
QH_PER_CORE = 4   # q heads per core (32/8)
QCOLS = QH_PER_CORE * HD      # 256 q-projection cols per core
KVCOLS = 2 * HD               # 128 packed k|v cols per core
ROWS_PER_CORE = S // N_CORES  # 256 output rows per core


def _build():
    nc = bacc.Bacc("TRN2", target_bir_lowering=False, debug=False,
                   num_devices=N_CORES)
    xT_d = nc.dram_tensor("xT", [D, S], F32R, kind="ExternalInput")
    wq_d = nc.dram_tensor("wq", [D, QCOLS], F32R, kind="ExternalInput")
    wkv_d = nc.dram_tensor("wkv", [D, KVCOLS], F32R, kind="ExternalInput")
    wo_d = nc.dram_tensor("wo", [D, D], F32R, kind="ExternalInput")
    cos_d = nc.dram_tensor("cosT", [HD // 2, S], F32, kind="ExternalInput")
    sin_d = nc.dram_tensor("sinT", [HD // 2, S], F32, kind="ExternalInput")
    mask_d = nc.dram_tensor("maskT", [128, 128], F32, kind="ExternalInput")
    out_d = nc.dram_tensor("out", [ROWS_PER_CORE, D], F32, kind="ExternalOutput")

    with tile.TileContext(nc) as tc, ExitStack() as top:
        persist = top.enter_context(tc.tile_pool(name="persist", bufs=1))
        qTs = [persist.tile([HD, S], F32R, name=f"qT{i}", uniquify=False)
               for i in range(QH_PER_CORE)]
        kT = persist.tile([HD, S], F32R, name="kT")
        v65 = persist.tile([128, 16, HD + 1], F32R, name="v65")
        attnT0 = persist.tile([128, S], F32R, name="attnT0")
        attnT1 = persist.tile([128, S], F32R, name="attnT1")
        attnTs = [attnT0, attnT1]
        maskT_sb = persist.tile([128, 128], F32, name="maskT_sb")
        nc.sync.dma_start(maskT_sb[:], mask_d.ap())

        dram = top.enter_context(tc.tile_pool(name="dram", bufs=1, space="DRAM"))
        a2a_in = dram.tile([N_CORES, ROWS_PER_CORE, ROWS_PER_CORE], F32R,
                           name="a2a_in")
        a2a_out = dram.tile([N_CORES, ROWS_PER_CORE, ROWS_PER_CORE], F32R,
                            name="a2a_out")

        # ---------------- Stage P: q/k/v projections + rope ----------------
        with ExitStack() as ctx:
            wpool = ctx.enter_context(tc.tile_pool(name="wpool", bufs=1))
            wq_sb = wpool.tile([128, 16, QCOLS], F32R, name="wq_sb")
            wkv_sb = wpool.tile([128, 16, KVCOLS], F32R, name="wkv_sb")
            cos_sb = wpool.tile([HD // 2, S], F32, name="cos_sb")
            sin_sb = wpool.tile([HD // 2, S], F32, name="sin_sb")
            vT = wpool.tile([HD + 1, S], F32, name="vT")
            identity = wpool.tile([HD + 1, HD + 1], F32, name="identity")
            make_identity(nc, identity[:])
            nc.sync.dma_start(wq_sb[:],
                              wq_d.ap().rearrange("(kc p) m -> p kc m", p=128))
            nc.sync.dma_start(wkv_sb[:],
                              wkv_d.ap().rearrange("(kc p) m -> p kc m", p=128))
            nc.sync.dma_start(cos_sb[:], cos_d.ap())
            nc.sync.dma_start(sin_sb[:], sin_d.ap())

            xt_pool = ctx.enter_context(tc.tile_pool(name="xt", bufs=3))
            pq_pool = ctx.enter_context(
                tc.tile_pool(name="pq", bufs=4, space="PSUM"))
            pkv_pool = ctx.enter_context(
                tc.tile_pool(name="pkv", bufs=2, space="PSUM"))
            pvt_pool = ctx.enter_context(
                tc.tile_pool(name="pvt", bufs=2, space="PSUM"))
            tmp_pool = ctx.enter_context(tc.tile_pool(name="ropetmp", bufs=8))

            def rope_pair(dst, dst_rows, src, a_rows, b_rows, cs, sn, tag):
                """dst[a] = a*cos - b*sin ; dst[b] = a*sin + b*cos."""
                a = src[a_rows[0]:a_rows[1], :]
                b = src[b_rows[0]:b_rows[1], :]
                t1 = tmp_pool.tile([32, 512], F32, name=f"t1{tag}", tag="t1")
                t2 = tmp_pool.tile([32, 512], F32, name=f"t2{tag}", tag="t2")
                nc.vector.tensor_mul(t1[:], a, cs)
                nc.vector.tensor_mul(t2[:], b, sn)
                nc.vector.tensor_sub(
                    dst[dst_rows[0]:dst_rows[0] + 32, dst_rows[2]:dst_rows[3]],
                    t1[:], t2[:])
                t3 = tmp_pool.tile([32, 512], F32, name=f"t3{tag}", tag="t3")
                t4 = tmp_pool.tile([32, 512], F32, name=f"t4{tag}", tag="t4")
                nc.vector.tensor_mul(t3[:], a, sn)
                nc.vector.tensor_mul(t4[:], b, cs)
                nc.vector.tensor_add(
                    dst[dst_rows[1]:dst_rows[1] + 32, dst_rows[2]:dst_rows[3]],
                    t3[:], t4[:])

            for sh in range(2):
                s0 = 1024 * sh
                pq = [[pq_pool.tile([128, 512], F32, name=f"pq{sh}_{m}{n}",
                                    tag="pq") for n in range(2)]
                      for m in range(2)]
                pkv = [pkv_pool.tile([128, 512], F32, name=f"pkv{sh}_{n}",
                                     tag="pkv") for n in range(2)]
                for kc in range(16):
                    xt = xt_pool.tile([128, 1024], F32R, name=f"xt{sh}_{kc}",
                                      tag="xt")
                    nc.sync.dma_start(
                        xt[:], xT_d.ap()[128 * kc:128 * (kc + 1), s0:s0 + 1024])
                    st, sp = (kc == 0), (kc == 15)
                    for m in range(2):
                        lhs = wq_sb[:, kc, 128 * m:128 * (m + 1)]
                        for n in range(2):
                            nc.tensor.matmul(
                                pq[m][n][:], lhs,
                                xt[:, 512 * n:512 * (n + 1)],
                                start=st, stop=sp)
                    lkv = wkv_sb[:, kc, :]
                    for n in range(2):
                        nc.tensor.matmul(
                            pkv[n][:], lkv,
                            xt[:, 512 * n:512 * (n + 1)],
                            start=st, stop=sp)
                # rope q -> qTs ; rope k -> kT ; copy v -> vT
                for n in range(2):
                    c0, c1 = s0 + 512 * n, s0 + 512 * (n + 1)
                    cs = cos_sb[:, c0:c1]
                    sn = sin_sb[:, c0:c1]
                    for m in range(2):
                        for hh in range(2):
                            r = 64 * hh
                            rope_pair(qTs[2 * m + hh], (0, 32, c0, c1),
                                      pq[m][n], (r, r + 32), (r + 32, r + 64),
                                      cs, sn, f"q{sh}{m}{n}{hh}")
                    rope_pair(kT, (0, 32, c0, c1), pkv[n], (0, 32), (32, 64),
                              cs, sn, f"k{sh}{n}")
                    nc.scalar.copy(vT[0:HD, c0:c1], pkv[n][64:128, :])

            # V65: transpose vT (65,S) -> natural (S,65) blocks; row 64 of vT
            # is all-ones so column 64 of V65 yields softmax denominators.
            nc.vector.memset(vT[HD:HD + 1, :], 1.0)
            for sc in range(16):
                pvt = pvt_pool.tile([128, HD + 1], F32, name=f"pvt{sc}",
                                    tag="pvt")
                nc.tensor.transpose(pvt[:], vT[:, 128 * sc:128 * (sc + 1)],
                                    identity[:])
                nc.scalar.copy(v65[:, sc, :], pvt[:])

        # ---------------- Stage A: attention per head / q-tile ----------------
        with ExitStack() as ctx:
            psc_pool = ctx.enter_context(
                tc.tile_pool(name="psc", bufs=3, space="PSUM"))
            po_pool = ctx.enter_context(
                tc.tile_pool(name="po", bufs=4, space="PSUM"))
            probs_pool = ctx.enter_context(tc.tile_pool(name="probs", bufs=4))
            nrm_pool = ctx.enter_context(tc.tile_pool(name="nrm", bufs=4))
            zf32 = nrm_pool.tile([128, 384], F32, name="zf32", bufs=1)
            nc.vector.memset(zf32[:], 0.0)
            zero_sb = nrm_pool.tile([128, 384], F32R, name="zero_sb", bufs=1)
            nc.vector.tensor_copy(zero_sb[:], zf32[:])
            for h in range(QH_PER_CORE):
                qh = qTs[h]
                ro = 0
                for t in range(4):
                    po = po_pool.tile([HD + 1, 512], F32, name=f"po{h}{t}",
                                      tag="po")
                    nb = 4 * t + 4
                    for b in range(nb):
                        j = max(0, b - 4 * t)
                        col0 = 128 * j
                        psc = psc_pool.tile([128, 512], F32,
                                            name=f"psc{h}{t}{b}", tag="psc")
                        nc.tensor.matmul(
                            psc[:, col0:512],
                            kT[:, 128 * b:128 * (b + 1)],
                            qh[:, 512 * t + col0:512 * (t + 1)],
                            start=True, stop=True)
                        if b >= 4 * t:  # diagonal 128x128 sub-block
                            nc.vector.tensor_add(psc[:, col0:col0 + 128],
                                                 psc[:, col0:col0 + 128],
                                                 maskT_sb[:])
                        probs = probs_pool.tile([128, 512], F32R,
                                                name=f"pr{h}{t}{b}",
                                                tag="probs")
                        nc.scalar.activation(probs[:, col0:512],
                                             psc[:, col0:512], AF.Exp,
                                             scale=0.125)
                        if col0:
                            nc.vector.tensor_copy(probs[:, 0:col0],
                                                  zero_sb[:, 0:col0])
                        nc.tensor.matmul(po[:], v65[:, b, :],
                                         probs[:],
                                         start=(b == 0), stop=(b == nb - 1))
                    recip = nrm_pool.tile([1, 512], F32, name=f"rc{h}{t}",
                                          tag="recip")
                    nc.vector.reciprocal(recip[:], po[HD:HD + 1, :])
                    rfac = nrm_pool.tile([HD, 512], F32, name=f"rf{h}{t}",
                                         tag="rfac")
                    nc.gpsimd.partition_broadcast(rfac[:], recip[:])
                    nc.vector.tensor_mul(
                        attnTs[h // 2][64 * (h % 2):64 * (h % 2) + HD,
                                       512 * t:512 * (t + 1)],
                        po[0:HD, :], rfac[:])

        # ---------------- AllToAll re-shard (heads -> seq rows) ----------------
        for r in range(N_CORES):
            nc.sync.dma_start(a2a_in[r, 0:128, :],
                              attnT0[:, 256 * r:256 * (r + 1)])
            nc.sync.dma_start(a2a_in[r, 128:256, :],
                              attnT1[:, 256 * r:256 * (r + 1)])
        nc.gpsimd.collective_compute(
            "AllToAll", mybir.AluOpType.bypass,
            replica_groups=[list(range(N_CORES))],
            ins=[a2a_in[:]], outs=[a2a_out[:]])

        # ---------------- Stage W: out rows = attn_fullT.T @ wo ----------------
        with ExitStack() as ctx:
            af_pool = ctx.enter_context(tc.tile_pool(name="af", bufs=1))
            attn_full = af_pool.tile([128, N_CORES, 2, ROWS_PER_CORE], F32R,
                                     name="attn_full")
            nc.sync.dma_start(
                attn_full[:],
                a2a_out[:].rearrange("r (h2 p) s -> p r h2 s", p=128))
            wo_pool = ctx.enter_context(tc.tile_pool(name="wop", bufs=3))
            pw_pool = ctx.enter_context(
                tc.tile_pool(name="pw", bufs=8, space="PSUM"))
            osb_pool = ctx.enter_context(tc.tile_pool(name="osb", bufs=2))
            pw = [[pw_pool.tile([128, 512], F32, name=f"pw{m}{n}", tag="pw")
                   for n in range(4)] for m in range(2)]
            for kc in range(16):
                wo_t = wo_pool.tile([128, D], F32R, name=f"wo{kc}", tag="wo")
                nc.sync.dma_start(wo_t[:], wo_d.ap()[128 * kc:128 * (kc + 1), :])
                st, sp = (kc == 0), (kc == 15)
                for m in range(2):
                    lhs = attn_full[:, kc // 2, kc % 2,
                                    128 * m:128 * (m + 1)]
                    for n in range(4):
                        nc.tensor.matmul(
                            pw[m][n][:], lhs,
                            wo_t[:, 512 * n:512 * (n + 1)],
                            start=st, stop=sp)
            for m in range(2):
                osb = osb_pool.tile([128, D], F32, name=f"osb{m}", tag="osb")
                for n in range(4):
                    nc.scalar.copy(osb[:, 512 * n:512 * (n + 1)], pw[m][n][:])
                nc.sync.dma_start(out_d.ap()[128 * m:128 * (m + 1), :], osb[:])

    nc.compile()
    return nc


_NC_CACHE = None
LAST_RESULT = None


def _get_nc():
    global _NC_CACHE
    if _NC_CACHE is None:
        _NC_CACHE = _build()
    return _NC_CACHE


def _permute_rope_cols(w):
    """Per-head column permutation: [d0,d1,...,d63] -> [evens..., odds...]."""
    Din, HDall = w.shape
    H = HDall // HD
    return np.ascontiguousarray(
        w.reshape(Din, H, HD // 2, 2).transpose(0, 1, 3, 2).reshape(Din, HDall))


def kernel(x, wq, wk, wv, wo, freqs_cos, freqs_sin, mask, start_pos=0):
    assert int(start_pos) == 0, "kernel specialized for start_pos == 0"
    x = np.asarray(x, np.float32)
    b, s, d = x.shape
    assert (b, s, d) == (1, S, D)
    xT = np.ascontiguousarray(x[0].T)
    wq_p = _permute_rope_cols(np.asarray(wq, np.float32))
    wk_p = _permute_rope_cols(np.asarray(wk, np.float32))
    wv = np.asarray(wv, np.float32)
    wo = np.ascontiguousarray(np.asarray(wo, np.float32))
    cosT = np.ascontiguousarray(np.asarray(freqs_cos, np.float32).T)
    sinT = np.ascontiguousarray(np.asarray(freqs_sin, np.float32).T)
    maskT = np.ascontiguousarray(np.asarray(mask, np.float32)[:128, :128].T)

    in_maps = []
    for c in range(N_CORES):
        in_maps.append({
            "xT": xT,
            "wq": np.ascontiguousarray(wq_p[:, QCOLS * c:QCOLS * (c + 1)]),
            "wkv": np.ascontiguousarray(np.concatenate(
                [wk_p[:, HD * c:HD * (c + 1)], wv[:, HD * c:HD * (c + 1)]],
                axis=1)),
            "wo": wo,
            "cosT": cosT,
            "sinT": sinT,
            "maskT": maskT,
        })

    nc = _get_nc()
    res = bass_utils.run_bass_kernel_spmd(
        nc, in_maps, core_ids=list(range(N_CORES)),
        trace=bool(os.environ.get("BASS_TRACE")))
    global LAST_RESULT
    LAST_RESULT = res
    rows = [res.results[c]["out"] for c in range(N_CORES)]
    return np.concatenate(rows, axis=0).reshape(1, S, D).astype(np.float32)


# revision 8
# speedup vs baseline: 1.2933x; 1.2933x over previous
"""GQA attention (S=2048, D=2048, 32 q-heads / 8 kv-heads, rope, causal) on 8
Trainium2 NeuronCores, tensor-parallel over heads (1 kv head + 4 q heads per
core), chunked AllToAll re-shard overlapped with compute, row-sharded output.

Self-contained: takes full inputs, shards on host, runs one SPMD Bass/Tile
kernel via run_bass_kernel_spmd, reassembles the full output.

Layout notes (activations on-chip live in the transposed/"T" domain):
 - xT (D,S) host-transposed so the contraction dim D is the SBUF partition dim.
 - q/k weights are column-permuted per head (evens then odds) so rope becomes
   ops on contiguous 32-row blocks; scores are permutation-invariant.
 - scoresT[s,q] = kT.T @ qT per 128-row s-block; softmax denominators come for
   free from a ones-row appended to vT (row 64 of the PV psum after transpose).
 - softmax skips the max-subtraction: scores*0.125 ~ N(0,1), exp is safe in f32.
 - causal masking: s-blocks strictly above the diagonal are skipped, the
   diagonal 128x128 sub-block gets mask[:128,:128].T added pre-exp (all
   diagonal blocks of a causal mask are identical), below-diagonal sub-block
   columns inside partial tiles are zero-filled in probs.
 - matmuls run in bf16 (fast weight load, fp32 psum accumulate); inputs are
   cast on the fly (gpsimd for xT tiles, vector for wo tiles).
"""
import os
import sys
from contextlib import ExitStack

import numpy as np

try:
    import concourse.bass as bass  # noqa: F401
except ImportError:  # platform tree not on sys.path in a fresh dir
    sys.path.insert(0, "/opt/trn_rl_repo")
    import concourse.bass as bass  # noqa: F401

import concourse.mybir as mybir
from concourse import bacc, bass_utils, tile
from concourse.masks import make_identity

F32 = mybir.dt.float32
BF16 = mybir.dt.bfloat16
AF = mybir.ActivationFunctionType

S = 2048          # sequence length
D = 2048          # model dim
HD = 64           # head dim
N_CORES = 8
QH_PER_CORE = 4   # q heads per core (32/8)
QCOLS = QH_PER_CORE * HD      # 256 q-projection cols per core
KVCOLS = 2 * HD               # 128 packed k|v cols per core
ROWS_PER_CORE = S // N_CORES  # 256 output rows per core


def _build():
    nc = bacc.Bacc("TRN2", target_bir_lowering=False, debug=False,
                   num_devices=N_CORES)
    xT_d = nc.dram_tensor("xT", [D, S], F32, kind="ExternalInput")
    wq_d = nc.dram_tensor("wq", [D, QCOLS], BF16, kind="ExternalInput")
    wkv_d = nc.dram_tensor("wkv", [D, KVCOLS], BF16, kind="ExternalInput")
    wo_d = nc.dram_tensor("wo", [D, D], F32, kind="ExternalInput")
    cos_d = nc.dram_tensor("cosT", [HD // 2, S], F32, kind="ExternalInput")
    sin_d = nc.dram_tensor("sinT", [HD // 2, S], F32, kind="ExternalInput")
    mask_d = nc.dram_tensor("maskT", [128, 128], F32, kind="ExternalInput")
    out_d = nc.dram_tensor("out", [ROWS_PER_CORE, D], F32, kind="ExternalOutput")

    with tile.TileContext(nc) as tc, ExitStack() as top:
        persist = top.enter_context(tc.tile_pool(name="persist", bufs=1))
        qTs = [persist.tile([HD, S], BF16, name=f"qT{i}", uniquify=False)
               for i in range(QH_PER_CORE)]
        kT = persist.tile([HD, S], BF16, name="kT")
        v128 = persist.tile([128, 16, 128], BF16, name="v128")
        attnT0 = persist.tile([128, S], BF16, name="attnT0")
        attnT1 = persist.tile([128, S], BF16, name="attnT1")
        attnTs = [attnT0, attnT1]
        maskT_sb = persist.tile([128, 128], F32, name="maskT_sb")
        nc.sync.dma_start(maskT_sb[:], mask_d.ap())
        # full wo prefetched + cast to bf16 during earlier stages
        wo_sb = persist.tile([128, 16, D], BF16, name="wo_sb")

        dram = top.enter_context(tc.tile_pool(name="dram", bufs=1, space="DRAM"))
        a2a_in = [dram.tile([N_CORES, 128, ROWS_PER_CORE], BF16,
                            name=f"a2a_in{i}", uniquify=False)
                  for i in range(2)]
        a2a_out = [dram.tile([N_CORES, 128, ROWS_PER_CORE], BF16,
                             name=f"a2a_out{i}", uniquify=False)
                   for i in range(2)]

        # ---------------- Stage P: q/k/v projections + rope ----------------
        with ExitStack() as ctx:
            wpool = ctx.enter_context(tc.tile_pool(name="wpool", bufs=1))
            wq_sb = wpool.tile([128, 16, QCOLS], BF16, name="wq_sb")
            wkv_sb = wpool.tile([128, 16, KVCOLS], BF16, name="wkv_sb")
            cos_sb = wpool.tile([HD // 2, S], F32, name="cos_sb")
            sin_sb = wpool.tile([HD // 2, S], F32, name="sin_sb")
            vT = wpool.tile([HD + 1, S], F32, name="vT")
            identity = wpool.tile([HD + 1, HD + 1], F32, name="identity")
            make_identity(nc, identity[:])
            nc.sync.dma_start(wq_sb[:],
                              wq_d.ap().rearrange("(kc p) m -> p kc m", p=128))
            nc.sync.dma_start(wkv_sb[:],
                              wkv_d.ap().rearrange("(kc p) m -> p kc m", p=128))
            nc.sync.dma_start(cos_sb[:], cos_d.ap())
            nc.sync.dma_start(sin_sb[:], sin_d.ap())

            xt_pool = ctx.enter_context(tc.tile_pool(name="xt", bufs=3))
            xtb_pool = ctx.enter_context(tc.tile_pool(name="xtb", bufs=3))
            pq_pool = ctx.enter_context(
                tc.tile_pool(name="pq", bufs=4, space="PSUM"))
            pkv_pool = ctx.enter_context(
                tc.tile_pool(name="pkv", bufs=2, space="PSUM"))
            pvt_pool = ctx.enter_context(
                tc.tile_pool(name="pvt", bufs=2, space="PSUM"))
            tmp_pool = ctx.enter_context(tc.tile_pool(name="ropetmp", bufs=2))

            def rope_pair(dst, dst_cols, src, a_row, cs, sn, tag):
                """dst rows [0:32] = a*cos - b*sin ; rows [32:64] = a*sin+b*cos
                with a = src rows [a_row:a_row+32], b = the next 32 rows."""
                a = src[a_row:a_row + 32, :]
                b = src[a_row + 32:a_row + 64, :]
                t1 = tmp_pool.tile([32, 512], F32, name=f"t1{tag}", tag="t1")
                t2 = tmp_pool.tile([32, 512], F32, name=f"t2{tag}", tag="t2")
                nc.vector.tensor_mul(t1[:], a, cs)
                nc.vector.tensor_mul(t2[:], b, sn)
                nc.vector.tensor_sub(
                    dst[0:32, dst_cols[0]:dst_cols[1]], t1[:], t2[:])
                t3 = tmp_pool.tile([32, 512], F32, name=f"t3{tag}", tag="t3")
                t4 = tmp_pool.tile([32, 512], F32, name=f"t4{tag}", tag="t4")
                nc.vector.tensor_mul(t3[:], a, sn)
                nc.vector.tensor_mul(t4[:], b, cs)
                nc.vector.tensor_add(
                    dst[32:64, dst_cols[0]:dst_cols[1]], t3[:], t4[:])

            for sq in range(4):
                s0 = 512 * sq
                pq = [pq_pool.tile([128, 512], F32, name=f"pq{sq}_{m}",
                                   tag="pq") for m in range(2)]
                pkv = pkv_pool.tile([128, 512], F32, name=f"pkv{sq}",
                                    tag="pkv")
                for kc in range(16):
                    xt = xt_pool.tile([128, 512], F32, name=f"xt{sq}_{kc}",
                                      tag="xt")
                    nc.sync.dma_start(
                        xt[:], xT_d.ap()[128 * kc:128 * (kc + 1), s0:s0 + 512])
                    xtb = xtb_pool.tile([128, 512], BF16,
                                        name=f"xtb{sq}_{kc}", tag="xtb")
                    nc.gpsimd.tensor_copy(xtb[:], xt[:])
                    st, sp = (kc == 0), (kc == 15)
                    for m in range(2):
                        nc.tensor.matmul(
                            pq[m][:], wq_sb[:, kc, 128 * m:128 * (m + 1)],
                            xtb[:], start=st, stop=sp)
                    nc.tensor.matmul(pkv[:], wkv_sb[:, kc, :], xtb[:],
                                     start=st, stop=sp)
                # rope q -> qTs ; rope k -> kT ; copy v -> vT
                cs = cos_sb[:, s0:s0 + 512]
                sn = sin_sb[:, s0:s0 + 512]
                for m in range(2):
                    for hh in range(2):
                        rope_pair(qTs[2 * m + hh], (s0, s0 + 512), pq[m],
                                  64 * hh, cs, sn, f"q{sq}{m}{hh}")
                rope_pair(kT, (s0, s0 + 512), pkv, 0, cs, sn, f"k{sq}")
                nc.scalar.copy(vT[0:HD, s0:s0 + 512], pkv[64:128, :])

            # prefetch + cast wo to bf16 (Tile overlaps this with the above)
            wof_pool = ctx.enter_context(tc.tile_pool(name="wof", bufs=2))
            for kc in range(16):
                wof = wof_pool.tile([128, D], F32, name=f"wof{kc}", tag="wof")
                nc.sync.dma_start(wof[:], wo_d.ap()[128 * kc:128 * (kc + 1), :])
                nc.vector.tensor_copy(wo_sb[:, kc, :], wof[:])

            # v128: transpose vT (65,S) -> natural (S,65) blocks; col 64 = ones
            # (softmax denominators); cols 65..127 zero (pad -> 128-wide
            # stationary operand enables the fast weight load path).
            nc.vector.memset(vT[HD:HD + 1, :], 1.0)
            nc.vector.memset(v128[:, :, HD + 1:], 0.0)
            for sc in range(16):
                pvt = pvt_pool.tile([128, HD + 1], F32, name=f"pvt{sc}",
                                    tag="pvt")
                nc.tensor.transpose(pvt[:], vT[:, 128 * sc:128 * (sc + 1)],
                                    identity[:])
                nc.scalar.copy(v128[:, sc, 0:HD + 1], pvt[:])

        # ---------------- Stage A + chunked A2A + Stage W ----------------
        with ExitStack() as ctx:
            psc_pool = ctx.enter_context(
                tc.tile_pool(name="psc", bufs=3, space="PSUM"))
            po_pool = ctx.enter_context(
                tc.tile_pool(name="po", bufs=3, space="PSUM"))
            probs_pool = ctx.enter_context(tc.tile_pool(name="probs", bufs=4))
            nrm_pool = ctx.enter_context(tc.tile_pool(name="nrm", bufs=4))
            zero_sb = nrm_pool.tile([128, 384], BF16, name="zero_sb", bufs=1)
            nc.vector.memset(zero_sb[:], 0.0)

            def attention_head(h):
                qh = qTs[h]
                for t in range(4):
                    po = po_pool.tile([128, 512], F32, name=f"po{h}{t}",
                                      tag="po")
                    nb = 4 * t + 4
                    for b in range(nb):
                        j = max(0, b - 4 * t)
                        col0 = 128 * j
                        psc = psc_pool.tile([128, 512], F32,
                                            name=f"psc{h}{t}{b}", tag="psc")
                        nc.tensor.matmul(
                            psc[:, col0:512],
                            kT[:, 128 * b:128 * (b + 1)],
                            qh[:, 512 * t + col0:512 * (t + 1)],
                            start=True, stop=True)
                        if b >= 4 * t:  # diagonal 128x128 sub-block
                            nc.vector.tensor_add(psc[:, col0:col0 + 128],
                                                 psc[:, col0:col0 + 128],
                                                 maskT_sb[:])
                        probs = probs_pool.tile([128, 512], BF16,
                                                name=f"pr{h}{t}{b}",
                                                tag="probs")
                        nc.scalar.activation(probs[:, col0:512],
                                             psc[:, col0:512], AF.Exp,
                                             scale=0.125)
                        if col0:
                            nc.vector.tensor_copy(probs[:, 0:col0],
                                                  zero_sb[:, 0:col0])
                        nc.tensor.matmul(po[:], v128[:, b, :], probs[:],
                                         start=(b == 0), stop=(b == nb - 1))
                    recip = nrm_pool.tile([1, 512], F32, name=f"rc{h}{t}",
                                          tag="recip")
                    nc.vector.reciprocal(recip[:], po[HD:HD + 1, :])
                    rfac = nrm_pool.tile([HD, 512], F32, name=f"rf{h}{t}",
                                         tag="rfac")
                    nc.gpsimd.partition_broadcast(rfac[:], recip[:])
                    nc.vector.tensor_mul(
                        attnTs[h // 2][64 * (h % 2):64 * (h % 2) + HD,
                                       512 * t:512 * (t + 1)],
                        po[0:HD, :], rfac[:])

            def send_a2a(i):
                for r in range(N_CORES):
                    nc.sync.dma_start(a2a_in[i][r],
                                      attnTs[i][:, 256 * r:256 * (r + 1)])
                nc.gpsimd.collective_compute(
                    "AllToAll", mybir.AluOpType.bypass,
                    replica_groups=[list(range(N_CORES))],
                    ins=[a2a_in[i][:]], outs=[a2a_out[i][:]])

            attention_head(0)
            attention_head(1)
            send_a2a(0)          # heads 0/1 shards move while 2/3 compute
            attention_head(2)
            attention_head(3)
            send_a2a(1)

        # Stage W: out rows = attn_fullT.T @ wo, accumulated in two passes
        # (even h-chunks from a2a chunk 0, odd from chunk 1).
        with ExitStack() as ctx:
            af_pool = ctx.enter_context(tc.tile_pool(name="af", bufs=1))
            pw_pool = ctx.enter_context(
                tc.tile_pool(name="pw", bufs=1, space="PSUM"))
            osb_pool = ctx.enter_context(tc.tile_pool(name="osb", bufs=2))
            afs = []
            for i in range(2):
                af = af_pool.tile([128, N_CORES, ROWS_PER_CORE], BF16,
                                  name=f"attn_full{i}", uniquify=False)
                nc.sync.dma_start(af[:],
                                  a2a_out[i][:].rearrange("r p s -> p r s"))
                afs.append(af)
            pw = [[pw_pool.tile([128, 512], F32, name=f"pw{m}{n}",
                                tag=f"pw{m}{n}") for n in range(4)]
                  for m in range(2)]
            for i in range(2):          # a2a chunk: even then odd h-chunks
                for r in range(N_CORES):
                    kc = 2 * r + i
                    st = (i == 0 and r == 0)
                    sp = (i == 1 and r == N_CORES - 1)
                    for m in range(2):
                        lhs = afs[i][:, r, 128 * m:128 * (m + 1)]
                        for n in range(4):
                            nc.tensor.matmul(
                                pw[m][n][:], lhs,
                                wo_sb[:, kc, 512 * n:512 * (n + 1)],
                                start=st, stop=sp)
            for m in range(2):
                osb = osb_pool.tile([128, D], F32, name=f"osb{m}", tag="osb")
                for n in range(4):
                    nc.scalar.copy(osb[:, 512 * n:512 * (n + 1)], pw[m][n][:])
                nc.sync.dma_start(out_d.ap()[128 * m:128 * (m + 1), :], osb[:])

    nc.compile()
    return nc


_NC_CACHE = None
LAST_RESULT = None


def _get_nc():
    global _NC_CACHE
    if _NC_CACHE is None:
        _NC_CACHE = _build()
    return _NC_CACHE


def _permute_rope_cols(w):
    """Per-head column permutation: [d0,d1,...,d63] -> [evens..., odds...]."""
    Din, HDall = w.shape
    H = HDall // HD
    return np.ascontiguousarray(
        w.reshape(Din, H, HD // 2, 2).transpose(0, 1, 3, 2).reshape(Din, HDall))


def kernel(x, wq, wk, wv, wo, freqs_cos, freqs_sin, mask, start_pos=0):
    assert int(start_pos) == 0, "kernel specialized for start_pos == 0"
    import ml_dtypes
    x = np.asarray(x, np.float32)
    b, s, d = x.shape
    assert (b, s, d) == (1, S, D)
    xT = np.ascontiguousarray(x[0].T)
    wq_p = _permute_rope_cols(np.asarray(wq, np.float32))
    wk_p = _permute_rope_cols(np.asarray(wk, np.float32))
    wv = np.asarray(wv, np.float32)
    wo = np.ascontiguousarray(np.asarray(wo, np.float32))
    cosT = np.ascontiguousarray(np.asarray(freqs_cos, np.float32).T)
    sinT = np.ascontiguousarray(np.asarray(freqs_sin, np.float32).T)
    maskT = np.ascontiguousarray(np.asarray(mask, np.float32)[:128, :128].T)

    in_maps = []
    for c in range(N_CORES):
        in_maps.append({
            "xT": xT,
            "wq": np.ascontiguousarray(
                wq_p[:, QCOLS * c:QCOLS * (c + 1)]).astype(ml_dtypes.bfloat16),
            "wkv": np.ascontiguousarray(np.concatenate(
                [wk_p[:, HD * c:HD * (c + 1)], wv[:, HD * c:HD * (c + 1)]],
                axis=1)).astype(ml_dtypes.bfloat16),
            "wo": wo,
            "cosT": cosT,
            "sinT": sinT,
            "maskT": maskT,
        })

    nc = _get_nc()
    res = bass_utils.run_bass_kernel_spmd(
        nc, in_maps, core_ids=list(range(N_CORES)),
        trace=bool(os.environ.get("BASS_TRACE")))
    global LAST_RESULT
    LAST_RESULT = res
    rows = [res.results[c]["out"] for c in range(N_CORES)]
    return np.concatenate(rows, axis=0).reshape(1, S, D).astype(np.float32)


# revision 10
# speedup vs baseline: 1.5588x; 1.2053x over previous
"""GQA attention (S=2048, D=2048, 32 q-heads / 8 kv-heads, rope, causal) on 8
Trainium2 NeuronCores, tensor-parallel over heads (1 kv head + 4 q heads per
core), chunked AllToAll re-shard overlapped with compute, row-sharded output.

Self-contained: takes full inputs, shards on host, runs one SPMD Bass/Tile
kernel via run_bass_kernel_spmd, reassembles the full output.

Layout notes (activations on-chip live in the transposed/"T" domain):
 - xT (D,S) host-transposed so the contraction dim D is the SBUF partition dim.
 - q/k weights are column-permuted per head (evens then odds) so rope becomes
   ops on contiguous 32-row blocks; scores are permutation-invariant.
 - scoresT[s,q] = kT.T @ qT per 128-row s-block; softmax denominators come for
   free from a ones-row appended to vT (row 64 of the PV psum after transpose).
 - softmax skips the max-subtraction: scores*0.125 ~ N(0,1), exp is safe in f32.
 - causal masking: s-blocks strictly above the diagonal are skipped, the
   diagonal 128x128 sub-block gets mask[:128,:128].T added pre-exp (all
   diagonal blocks of a causal mask are identical), below-diagonal sub-block
   columns inside partial tiles are zero-filled in probs.
 - matmuls run in bf16 (fast weight load, fp32 psum accumulate); inputs are
   cast on the fly (gpsimd for xT tiles, vector for wo tiles).
"""
import os
import sys
from contextlib import ExitStack

import numpy as np

try:
    import concourse.bass as bass  # noqa: F401
except ImportError:  # platform tree not on sys.path in a fresh dir
    sys.path.insert(0, "/opt/trn_rl_repo")
    import concourse.bass as bass  # noqa: F401

import concourse.mybir as mybir
from concourse import bacc, bass_utils, tile
from concourse.masks import make_identity

F32 = mybir.dt.float32
BF16 = mybir.dt.bfloat16
AF = mybir.ActivationFunctionType

S = 2048          # sequence length
D = 2048          # model dim
HD = 64           # head dim
N_CORES = 8
QH_PER_CORE = 4   # q heads per core (32/8)
QCOLS = QH_PER_CORE * HD      # 256 q-projection cols per core
KVCOLS = 2 * HD               # 128 packed k|v cols per core
ROWS_PER_CORE = S // N_CORES  # 256 output rows per core


def _build():
    nc = bacc.Bacc("TRN2", target_bir_lowering=False, debug=False,
                   num_devices=N_CORES)
    xT_d = nc.dram_tensor("xT", [D, S], BF16, kind="ExternalInput")
    wq_d = nc.dram_tensor("wq", [D, QCOLS], BF16, kind="ExternalInput")
    wkv_d = nc.dram_tensor("wkv", [D, KVCOLS], BF16, kind="ExternalInput")
    wo_d = nc.dram_tensor("wo", [D, D], BF16, kind="ExternalInput")
    cos_d = nc.dram_tensor("cosT", [HD // 2, S], F32, kind="ExternalInput")
    sin_d = nc.dram_tensor("sinT", [HD // 2, S], F32, kind="ExternalInput")
    mask_d = nc.dram_tensor("maskT", [128, 128], F32, kind="ExternalInput")
    out_d = nc.dram_tensor("out", [ROWS_PER_CORE, D], F32, kind="ExternalOutput")

    with tile.TileContext(nc) as tc, ExitStack() as top:
        persist = top.enter_context(tc.tile_pool(name="persist", bufs=1))
        qTs = [persist.tile([HD, S], BF16, name=f"qT{i}", uniquify=False)
               for i in range(QH_PER_CORE)]
        kT = persist.tile([HD, S], BF16, name="kT")
        v128 = persist.tile([128, 16, 128], BF16, name="v128")
        attnT0 = persist.tile([128, S], BF16, name="attnT0")
        attnT1 = persist.tile([128, S], BF16, name="attnT1")
        attnTs = [attnT0, attnT1]
        maskT_sb = persist.tile([128, 128], F32, name="maskT_sb")
        nc.sync.dma_start(maskT_sb[:], mask_d.ap())
        # full wo prefetched + cast to bf16 during earlier stages
        wo_sb = persist.tile([128, 16, D], BF16, name="wo_sb")

        dram = top.enter_context(tc.tile_pool(name="dram", bufs=1, space="DRAM"))
        a2a_in = [dram.tile([N_CORES, 128, ROWS_PER_CORE], BF16,
                            name=f"a2a_in{i}", uniquify=False)
                  for i in range(2)]
        a2a_out = [dram.tile([N_CORES, 128, ROWS_PER_CORE], BF16,
                             name=f"a2a_out{i}", uniquify=False)
                   for i in range(2)]

        # ---------------- Stage P: q/k/v projections + rope ----------------
        with ExitStack() as ctx:
            wpool = ctx.enter_context(tc.tile_pool(name="wpool", bufs=1))
            wq_sb = wpool.tile([128, 16, QCOLS], BF16, name="wq_sb")
            wkv_sb = wpool.tile([128, 16, KVCOLS], BF16, name="wkv_sb")
            cos_sb = wpool.tile([HD // 2, S], F32, name="cos_sb")
            sin_sb = wpool.tile([HD // 2, S], F32, name="sin_sb")
            vT = wpool.tile([HD + 1, S], F32, name="vT")
            identity = wpool.tile([HD + 1, HD + 1], F32, name="identity")
            make_identity(nc, identity[:])
            nc.sync.dma_start(wq_sb[:],
                              wq_d.ap().rearrange("(kc p) m -> p kc m", p=128))
            nc.sync.dma_start(wkv_sb[:],
                              wkv_d.ap().rearrange("(kc p) m -> p kc m", p=128))
            nc.sync.dma_start(cos_sb[:], cos_d.ap())
            nc.sync.dma_start(sin_sb[:], sin_d.ap())
            nc.sync.dma_start(wo_sb[:],
                              wo_d.ap().rearrange("(kc p) m -> p kc m", p=128))

            xtb_pool = ctx.enter_context(tc.tile_pool(name="xtb", bufs=6))
            pq_pool = ctx.enter_context(
                tc.tile_pool(name="pq", bufs=4, space="PSUM"))
            pkv_pool = ctx.enter_context(
                tc.tile_pool(name="pkv", bufs=2, space="PSUM"))
            pvt_pool = ctx.enter_context(
                tc.tile_pool(name="pvt", bufs=2, space="PSUM"))
            tmp_pool = ctx.enter_context(tc.tile_pool(name="ropetmp", bufs=2))

            def rope_pair(dst, dst_cols, src, a_row, cs, sn, tag):
                """dst rows [0:32] = a*cos - b*sin ; rows [32:64] = a*sin+b*cos
                with a = src rows [a_row:a_row+32], b = the next 32 rows."""
                a = src[a_row:a_row + 32, :]
                b = src[a_row + 32:a_row + 64, :]
                t1 = tmp_pool.tile([32, 512], F32, name=f"t1{tag}", tag="t1")
                t2 = tmp_pool.tile([32, 512], F32, name=f"t2{tag}", tag="t2")
                nc.vector.tensor_mul(t1[:], a, cs)
                nc.vector.tensor_mul(t2[:], b, sn)
                nc.vector.tensor_sub(
                    dst[0:32, dst_cols[0]:dst_cols[1]], t1[:], t2[:])
                t3 = tmp_pool.tile([32, 512], F32, name=f"t3{tag}", tag="t3")
                t4 = tmp_pool.tile([32, 512], F32, name=f"t4{tag}", tag="t4")
                nc.vector.tensor_mul(t3[:], a, sn)
                nc.vector.tensor_mul(t4[:], b, cs)
                nc.vector.tensor_add(
                    dst[32:64, dst_cols[0]:dst_cols[1]], t3[:], t4[:])

            for sq in range(4):
                s0 = 512 * sq
                pq = [pq_pool.tile([128, 512], F32, name=f"pq{sq}_{m}",
                                   tag="pq") for m in range(2)]
                pkv = pkv_pool.tile([128, 512], F32, name=f"pkv{sq}",
                                    tag="pkv")
                for kc in range(16):
                    xtb = xtb_pool.tile([128, 512], BF16,
                                        name=f"xtb{sq}_{kc}", tag="xtb")
                    nc.sync.dma_start(
                        xtb[:], xT_d.ap()[128 * kc:128 * (kc + 1), s0:s0 + 512])
                    st, sp = (kc == 0), (kc == 15)
                    for m in range(2):
                        nc.tensor.matmul(
                            pq[m][:], wq_sb[:, kc, 128 * m:128 * (m + 1)],
                            xtb[:], start=st, stop=sp)
                    nc.tensor.matmul(pkv[:], wkv_sb[:, kc, :], xtb[:],
                                     start=st, stop=sp)
                # rope q -> qTs ; rope k -> kT ; copy v -> vT
                cs = cos_sb[:, s0:s0 + 512]
                sn = sin_sb[:, s0:s0 + 512]
                for m in range(2):
                    for hh in range(2):
                        rope_pair(qTs[2 * m + hh], (s0, s0 + 512), pq[m],
                                  64 * hh, cs, sn, f"q{sq}{m}{hh}")
                rope_pair(kT, (s0, s0 + 512), pkv, 0, cs, sn, f"k{sq}")
                nc.scalar.copy(vT[0:HD, s0:s0 + 512], pkv[64:128, :])


            # v128: transpose vT (65,S) -> natural (S,65) blocks; col 64 = ones
            # (softmax denominators); cols 65..127 zero (pad -> 128-wide
            # stationary operand enables the fast weight load path).
            nc.vector.memset(vT[HD:HD + 1, :], 1.0)
            nc.vector.memset(v128[:, :, HD + 1:], 0.0)
            for sc in range(16):
                pvt = pvt_pool.tile([128, HD + 1], F32, name=f"pvt{sc}",
                                    tag="pvt")
                nc.tensor.transpose(pvt[:], vT[:, 128 * sc:128 * (sc + 1)],
                                    identity[:])
                nc.scalar.copy(v128[:, sc, 0:HD + 1], pvt[:])

        # ---------------- Stage A + chunked A2A + Stage W ----------------
        with ExitStack() as ctx:
            psc_pool = ctx.enter_context(
                tc.tile_pool(name="psc", bufs=4, space="PSUM"))
            po_pool = ctx.enter_context(
                tc.tile_pool(name="po", bufs=3, space="PSUM"))
            probs_pool = ctx.enter_context(tc.tile_pool(name="probs", bufs=6))
            nrm_pool = ctx.enter_context(tc.tile_pool(name="nrm", bufs=4))

            def attention_head(h):
                qh = qTs[h]
                for t in range(4):
                    po = po_pool.tile([128, 512], F32, name=f"po{h}{t}",
                                      tag="po")
                    nb = 4 * t + 4
                    for b in range(nb):
                        j = max(0, b - 4 * t)
                        col0 = 128 * j
                        psc = psc_pool.tile([128, 512], F32,
                                            name=f"psc{h}{t}{b}", tag="psc")
                        nc.tensor.matmul(
                            psc[:, col0:512],
                            kT[:, 128 * b:128 * (b + 1)],
                            qh[:, 512 * t + col0:512 * (t + 1)],
                            start=True, stop=True)
                        if b >= 4 * t:  # diagonal 128x128 sub-block
                            nc.vector.tensor_add(psc[:, col0:col0 + 128],
                                                 psc[:, col0:col0 + 128],
                                                 maskT_sb[:])
                        probs = probs_pool.tile([128, 512], BF16,
                                                name=f"pr{h}{t}{b}",
                                                tag="probs")
                        nc.scalar.activation(probs[:, col0:512],
                                             psc[:, col0:512], AF.Exp,
                                             scale=0.125)
                        if col0:
                            nc.gpsimd.memset(probs[:, 0:col0], 0.0)
                        nc.tensor.matmul(po[:], v128[:, b, :], probs[:],
                                         start=(b == 0), stop=(b == nb - 1))
                    recip = nrm_pool.tile([1, 512], F32, name=f"rc{h}{t}",
                                          tag="recip")
                    nc.vector.reciprocal(recip[:], po[HD:HD + 1, :])
                    rfac = nrm_pool.tile([HD, 512], F32, name=f"rf{h}{t}",
                                         tag="rfac")
                    nc.gpsimd.partition_broadcast(rfac[:], recip[:])
                    nc.vector.tensor_mul(
                        attnTs[h // 2][64 * (h % 2):64 * (h % 2) + HD,
                                       512 * t:512 * (t + 1)],
                        po[0:HD, :], rfac[:])

            def send_a2a(i):
                for r in range(N_CORES):
                    nc.sync.dma_start(a2a_in[i][r],
                                      attnTs[i][:, 256 * r:256 * (r + 1)])
                nc.gpsimd.collective_compute(
                    "AllToAll", mybir.AluOpType.bypass,
                    replica_groups=[list(range(N_CORES))],
                    ins=[a2a_in[i][:]], outs=[a2a_out[i][:]])

            attention_head(0)
            attention_head(1)
            send_a2a(0)          # heads 0/1 shards move while 2/3 compute
            attention_head(2)
            attention_head(3)
            send_a2a(1)

        # Stage W: out rows = attn_fullT.T @ wo, accumulated in two passes
        # (even h-chunks from a2a chunk 0, odd from chunk 1).
        with ExitStack() as ctx:
            af_pool = ctx.enter_context(tc.tile_pool(name="af", bufs=1))
            pw_pool = ctx.enter_context(
                tc.tile_pool(name="pw", bufs=1, space="PSUM"))
            osb_pool = ctx.enter_context(tc.tile_pool(name="osb", bufs=2))
            afs = []
            for i in range(2):
                af = af_pool.tile([128, N_CORES, ROWS_PER_CORE], BF16,
                                  name=f"attn_full{i}", uniquify=False)
                nc.sync.dma_start(af[:],
                                  a2a_out[i][:].rearrange("r p s -> p r s"))
                afs.append(af)
            pw = [[pw_pool.tile([128, 512], F32, name=f"pw{m}{n}",
                                tag=f"pw{m}{n}") for n in range(4)]
                  for m in range(2)]
            for i in range(2):          # a2a chunk: even then odd h-chunks
                for r in range(N_CORES):
                    kc = 2 * r + i
                    st = (i == 0 and r == 0)
                    sp = (i == 1 and r == N_CORES - 1)
                    for m in range(2):
                        lhs = afs[i][:, r, 128 * m:128 * (m + 1)]
                        for n in range(4):
                            nc.tensor.matmul(
                                pw[m][n][:], lhs,
                                wo_sb[:, kc, 512 * n:512 * (n + 1)],
                                start=st, stop=sp)
            for m in range(2):
                osb = osb_pool.tile([128, D], F32, name=f"osb{m}", tag="osb")
                for n in range(4):
                    nc.scalar.copy(osb[:, 512 * n:512 * (n + 1)], pw[m][n][:])
                nc.sync.dma_start(out_d.ap()[128 * m:128 * (m + 1), :], osb[:])

    nc.compile()
    return nc


_NC_CACHE = None
LAST_RESULT = None


def _get_nc():
    global _NC_CACHE
    if _NC_CACHE is None:
        _NC_CACHE = _build()
    return _NC_CACHE


def _permute_rope_cols(w):
    """Per-head column permutation: [d0,d1,...,d63] -> [evens..., odds...]."""
    Din, HDall = w.shape
    H = HDall // HD
    return np.ascontiguousarray(
        w.reshape(Din, H, HD // 2, 2).transpose(0, 1, 3, 2).reshape(Din, HDall))


def kernel(x, wq, wk, wv, wo, freqs_cos, freqs_sin, mask, start_pos=0):
    assert int(start_pos) == 0, "kernel specialized for start_pos == 0"
    import ml_dtypes
    x = np.asarray(x, np.float32)
    b, s, d = x.shape
    assert (b, s, d) == (1, S, D)
    xT = np.ascontiguousarray(x[0].T).astype(ml_dtypes.bfloat16)
    wq_p = _permute_rope_cols(np.asarray(wq, np.float32))
    wk_p = _permute_rope_cols(np.asarray(wk, np.float32))
    wv = np.asarray(wv, np.float32)
    wo = np.ascontiguousarray(np.asarray(wo, np.float32)).astype(
        ml_dtypes.bfloat16)
    cosT = np.ascontiguousarray(np.asarray(freqs_cos, np.float32).T)
    sinT = np.ascontiguousarray(np.asarray(freqs_sin, np.float32).T)
    maskT = np.ascontiguousarray(np.asarray(mask, np.float32)[:128, :128].T)

    in_maps = []
    for c in range(N_CORES):
        in_maps.append({
            "xT": xT,
            "wq": np.ascontiguousarray(
                wq_p[:, QCOLS * c:QCOLS * (c + 1)]).astype(ml_dtypes.bfloat16),
            "wkv": np.ascontiguousarray(np.concatenate(
                [wk_p[:, HD * c:HD * (c + 1)], wv[:, HD * c:HD * (c + 1)]],
                axis=1)).astype(ml_dtypes.bfloat16),
            "wo": wo,
            "cosT": cosT,
            "sinT": sinT,
            "maskT": maskT,
        })

    nc = _get_nc()
    res = bass_utils.run_bass_kernel_spmd(
        nc, in_maps, core_ids=list(range(N_CORES)),
        trace=bool(os.environ.get("BASS_TRACE")))
    global LAST_RESULT
    LAST_RESULT = res
    rows = [res.results[c]["out"] for c in range(N_CORES)]
    return np.concatenate(rows, axis=0).reshape(1, S, D).astype(np.float32)
